# revision 15
# baseline (speedup 1.0000x reference)
"""Causal self-attention for B=4, L=2048, D=768, H=6 on 8 TRN2 NeuronCores.

Sharding: 8 cores = 4 batches x 2 head-groups (3 heads / 384 hidden each).
All matmul operands are fp16 (host converts x/weights; ~0.7% rel err, well
inside the 2e-2 gate). Per core, for its (batch, head-group):

  x^T is uploaded pre-transposed (fp16), so no PE transposes at all.
  QT/KT = (Wq,k chunk)^T-stationary @ x^T-moving   [128d x L per head]
  V     = x^T-stationary @ Wv-moving               [L x 384]
  per head, per 512-wide q-group, per 128-key block (causal skip at 128
  granularity — fp16 runs 1 cyc/row at any width):
    S^T  = K_blk @ Q^T            (PE)
    A^T  = exp(S^T/sqrt(128) - 2) (ACT, fp16 out; -2 guards fp16 range)
    tri-mask on diagonal blocks   (DVE, fp16 2x mode)
    O^T += V_blk^T @ A^T          (PE, accumulated in PSUM)
    Bsum += A^T                   (DVE fp16 adds — replaces the row-sum
                                   ones-matmuls that used to burn PE time)
  sums  = partition_all_reduce(Bsum)  (GPSIMD/Pool for groups 0-2; the g3
          streams use a PE ones-matmul: shorter latency at the tail)
  recip = 1/sums; O^T *= recip        (DVE)
  Y_part = O @ Wo_slice               (PE, via O^T-stationary)

Scheduling: QKV group g+1 is interleaved between group g's attention
batches so exp (ACT) latency hides behind projection matmuls, and the two
largest q-groups (2 and 3) run their per-head attention streams zipped
pairwise so the exp-heavy tail still has PE filler.  Host sums the two
head-group partials per batch and adds (bv @ Wo + bo); bq/bk are applied
on-device. The exp -2 bias cancels in softmax normalization exactly.
"""

import math

import numpy as np

import concourse.bacc as bacc
import concourse.mybir as mybir
import concourse.tile as tile
from concourse import bass_isa
from concourse.bass_utils import run_bass_kernel_spmd

F32 = mybir.dt.float32
F16 = mybir.dt.float16
EXP = mybir.ActivationFunctionType.Exp
IDENT = mybir.ActivationFunctionType.Identity

B = 4
L = 2048
D = 768
HEADS = 6
HD = 128
HPC = 3          # heads per core
HG = HPC * HD    # 384: per-core slice of the hidden dim
CB = D // 128    # 6 contraction chunks
SCALE = 1.0 / math.sqrt(HD)
EXP_BIAS = -2.0  # exp(S*scale - 2): keeps A and its sums in fp16 range
N_CORES = 8


def _merge_streams(a, bb):
    """Interleave two batch lists by fractional progress (b first on ties)."""
    tagged = [((i + 1) / len(bb), 1, x) for i, x in enumerate(bb)]
    tagged += [((i + 1) / len(a), 2, x) for i, x in enumerate(a)]
    tagged.sort(key=lambda t: (t[0], t[1]))
    return [x for _, _, x in tagged]


def build_nc(L_=L):
    """Build + compile the per-core Bass program (same program on all cores)."""
    NQG = L_ // 512   # 512-wide q groups

    nc = bacc.Bacc("TRN2", target_bir_lowering=False, debug=False)
    xt_d = nc.dram_tensor("xt", [D, L_], F16, kind="ExternalInput").ap()
    wqkv_d = nc.dram_tensor("wqkv", [D, 3 * HG], F16, kind="ExternalInput").ap()
    wo_d = nc.dram_tensor("wo", [HG, D], F16, kind="ExternalInput").ap()
    bq_d = nc.dram_tensor("bq", [HG], F32, kind="ExternalInput").ap()
    bk_d = nc.dram_tensor("bk", [HG], F32, kind="ExternalInput").ap()
    tri_d = nc.dram_tensor("tri", [128, 256], F16, kind="ExternalInput").ap()
    eb_d = nc.dram_tensor("eb", [128, 1], F32, kind="ExternalInput").ap()
    y_d = nc.dram_tensor("y", [L_, D], F16, kind="ExternalOutput").ap()

    with tile.TileContext(nc) as tc:
        with (
            tc.tile_pool(name="persist", bufs=1) as pp,
            tc.tile_pool(name="qkv_sb", bufs=1) as pqkv,
            tc.tile_pool(name="at_pool", bufs=8) as pat,
            tc.tile_pool(name="bsum_p", bufs=3) as pbs,
            tc.tile_pool(name="nrm_sb", bufs=3) as pn,
            tc.tile_pool(name="y_pool", bufs=3) as py_,
            tc.tile_pool(name="ps_s", bufs=2, space="PSUM") as ps_s,
            tc.tile_pool(name="ps_o", bufs=2, space="PSUM") as ps_o,
            tc.tile_pool(name="ps_sh", bufs=2, space="PSUM") as ps_sh,
        ):
            # tiny memset-fed matmul right at program start: begins the PE
            # p-state ramp clock ~3us before the first real matmul, so QKV
            # group 0 runs at full clock
            dseed = pp.tile([128, 2], F16)
            nc.vector.memset(dseed, 0)
            dmy = ps_sh.tile([128, 512], F32, tag="sh", name="dmy")
            nc.tensor.matmul(
                dmy[:1, :2], dseed[:, :1], dseed, start=True, stop=True
            )

            # constants go on the SWDGE (gpsimd) queue so the HWDGE queue's
            # first descriptors are the weight / x^T chunks the PE waits on
            eb = pp.tile([128, 1], F32)
            nc.gpsimd.dma_start(eb, eb_d)
            trio = pp.tile([128, 256], F16)
            nc.gpsimd.dma_start(trio, tri_d)
            bq_sb = pp.tile([128, HPC], F32)
            bk_sb = pp.tile([128, HPC], F32)
            nc.gpsimd.dma_start(bq_sb, bq_d.rearrange("(h p) -> p h", p=128))
            nc.gpsimd.dma_start(bk_sb, bk_d.rearrange("(h p) -> p h", p=128))
            # dummy exp: pulls the ACT Exp-table load off the critical path
            warm = pp.tile([1, 1], F32)
            nc.scalar.activation(warm, eb[:1, :], EXP, bias=eb[:1, :])

            q_t = pqkv.tile([128, HPC, L_], F16)   # Q^T: [d, (head, L)]
            k_t = pqkv.tile([128, HPC, L_], F16)   # K^T
            v_t = pqkv.tile([128, L_ // 128, HG], F16)  # V: [k-in-block, (block, hd)]
            o_t = pqkv.tile([128, HPC, L_], F16)   # O^T (normalized)
            xt = pqkv.tile([128, CB, L_], F16)     # x^T: [d-in-chunk, (chunk, L)]
            wqkv_sb = pqkv.tile([128, CB, 3 * HG], F16)
            wo_sb = pqkv.tile([128, HPC, D], F16)

            xt_r = xt_d.rearrange("(c p) l -> p c l", p=128)
            wqkv_r = wqkv_d.rearrange("(c p) d -> p c d", p=128)
            # stage the uploads in consumption order: group-0 operands first
            # (weight chunk + first-512 x^T columns per chunk), then the
            # remaining x^T columns one 512-group at a time
            nc.sync.dma_start(wqkv_sb[:, 0, 0:HG], wqkv_r[:, 0, 0:HG])
            nc.sync.dma_start(xt[:, 0, 0:512], xt_r[:, 0, 0:512])
            nc.sync.dma_start(wqkv_sb[:, 0, HG:], wqkv_r[:, 0, HG:])
            for c in range(1, CB):
                nc.sync.dma_start(wqkv_sb[:, c, :], wqkv_r[:, c, :])
                nc.sync.dma_start(xt[:, c, 0:512], xt_r[:, c, 0:512])
            for g in range(1, NQG):
                qsl = slice(g * 512, (g + 1) * 512)
                # one DMA covers all six chunks for the group: a single
                # HWDGE pass instead of six serialized ones
                nc.sync.dma_start(xt[:, :, qsl], xt_r[:, :, qsl])
                if g == 1:
                    nc.sync.dma_start(
                        wo_sb, wo_d.rearrange("(h p) e -> p h e", p=128)
                    )

            tri = trio[:, 0:128]
            ones = trio[:, 128:256]

            # ---- QKV group 0: chunk-major so the PE rides the arriving
            # per-chunk DMAs without stalling (3 heads' q+k accumulate in
            # 6 PSUM banks at once; attention pools are idle this early) ----
            qk_ps = {
                0: (lambda t_: (t_[:, 0, :], t_[:, 1, :]))(
                    ps_s.tile([128, 2, 512], F32, tag="ps", name="qk0")
                ),
                1: (lambda t_: (t_[:, 0, :], t_[:, 1, :]))(
                    ps_s.tile([128, 2, 512], F32, tag="ps", name="qk1")
                ),
                2: (
                    ps_o.tile([128, 512], F32, tag="po", name="qk2q"),
                    ps_sh.tile([128, 512], F32, tag="sh", name="qk2k"),
                ),
            }
            for c in range(CB):
                for h in range(HPC):
                    pq, pk = qk_ps[h]
                    nc.tensor.matmul(
                        pq, wqkv_sb[:, c, h * 128 : (h + 1) * 128],
                        xt[:, c, 0:512],
                        start=(c == 0), stop=(c == CB - 1),
                    )
                    nc.tensor.matmul(
                        pk, wqkv_sb[:, c, HG + h * 128 : HG + (h + 1) * 128],
                        xt[:, c, 0:512],
                        start=(c == 0), stop=(c == CB - 1),
                    )
            for h in range(HPC):
                pq, pk = qk_ps[h]
                nc.scalar.activation(
                    q_t[:, h, 0:512], pq, IDENT, bias=bq_sb[:, h : h + 1]
                )
                nc.scalar.activation(
                    k_t[:, h, 0:512], pk, IDENT, bias=bk_sb[:, h : h + 1]
                )
            for b in range(4):
                pv = ps_sh.tile([128, 512], F32, tag="sh", name="pv")
                for c in range(CB):
                    nc.tensor.matmul(
                        pv[:, :HG], xt[:, c, b * 128 : (b + 1) * 128],
                        wqkv_sb[:, c, 2 * HG : 3 * HG],
                        start=(c == 0), stop=(c == CB - 1),
                    )
                nc.vector.tensor_copy(v_t[:, b, :], pv[:, :HG])

            # ---- interleaved QKV / attention stream ----

            def emit_qk_unit(g, h):
                qsl = slice(g * 512, (g + 1) * 512)
                pq = ps_sh.tile([128, 512], F32, tag="sh", name="pq")
                for c in range(CB):
                    nc.tensor.matmul(
                        pq, wqkv_sb[:, c, h * 128 : (h + 1) * 128], xt[:, c, qsl],
                        start=(c == 0), stop=(c == CB - 1),
                    )
                nc.scalar.activation(
                    q_t[:, h, qsl], pq, IDENT, bias=bq_sb[:, h : h + 1]
                )
                pk = ps_sh.tile([128, 512], F32, tag="sh", name="pk")
                for c in range(CB):
                    nc.tensor.matmul(
                        pk, wqkv_sb[:, c, HG + h * 128 : HG + (h + 1) * 128],
                        xt[:, c, qsl],
                        start=(c == 0), stop=(c == CB - 1),
                    )
                nc.scalar.activation(
                    k_t[:, h, qsl], pk, IDENT, bias=bk_sb[:, h : h + 1]
                )

            def emit_v_unit(g, b):
                lb = g * 4 + b
                pv = ps_sh.tile([128, 512], F32, tag="sh", name="pv")
                for c in range(CB):
                    nc.tensor.matmul(
                        pv[:, :HG], xt[:, c, lb * 128 : (lb + 1) * 128],
                        wqkv_sb[:, c, 2 * HG : 3 * HG],
                        start=(c == 0), stop=(c == CB - 1),
                    )
                nc.vector.tensor_copy(v_t[:, lb, :], pv[:, :HG])

            # ---- batch order: groups 0/1 sequential, groups 2/3 zipped ----
            batch_order = []
            fill = {}

            def add_fill(idx, u):
                fill.setdefault(idx, []).append(u)

            for g in range(NQG):
                units = []
                if g + 1 < NQG:
                    units = [("qk", g + 1, h) for h in range(HPC)]
                    units += [("v", g + 1, b) for b in range(4)]
                nb = 6 * (g + 1)
                base = len(batch_order)
                for h in range(HPC):
                    for j in range(2 * (g + 1)):
                        batch_order.append((g, h, j))
                for i, u in enumerate(units):
                    # unit i goes just before batch base + floor(i*nb/len)
                    add_fill(base + (i * nb) // len(units), u)

            last_j = {}
            first_j = {}
            for g in range(NQG):
                for h in range(HPC):
                    last_j[(g, h)] = 2 * (g + 1) - 1
                    first_j[(g, h)] = 0

            state = {}
            pending = []  # (delay, closure)

            def emit_S(gg, hh, jj):
                ps = ps_s.tile([128, 2, 512], F32, tag="ps")
                for t in range(2):
                    kb = 2 * jj + t
                    i = kb - 4 * gg
                    c0 = 128 * i if i > 0 else 0
                    nc.tensor.matmul(
                        ps[:, t, c0:],
                        k_t[:, hh, kb * 128 : (kb + 1) * 128],
                        q_t[:, hh, gg * 512 + c0 : (gg + 1) * 512],
                        start=True, stop=True,
                    )
                state[(gg, hh, jj)] = ps

            def emit_finalize(g, h):
                def run():
                    po = state.pop(("po", g, h))
                    state.pop(("bs", g, h))
                    sums = state.pop(("sm", g, h))
                    recip = pn.tile([128, 512], F32, tag="recip")
                    nc.vector.reciprocal(recip, sums)
                    nc.vector.tensor_mul(
                        o_t[:, h, g * 512 : (g + 1) * 512], po, recip
                    )
                return run

            def emit_rest(m):
                g, h, j = m
                ps = state.pop(m)
                last = j == last_j[(g, h)]
                first = j == first_j[(g, h)]
                if first:
                    state[("po", g, h)] = ps_o.tile(
                        [128, 512], F32, tag="po", name="po"
                    )
                    state[("bs", g, h)] = pbs.tile(
                        [128, 512], F16, tag="bs", name="bsum"
                    )
                po = state[("po", g, h)]
                bsum = state[("bs", g, h)]
                at = pat.tile([128, 2, 512], F16)
                diag = j >= 2 * g
                if diag:
                    # single strided call over both key blocks, starting at
                    # the first block's causal offset; the [c0a, c0b) sliver
                    # of t=1 exp's stale PSUM that nothing ever reads
                    c0a = 128 * (2 * j - 4 * g)
                    nc.scalar.activation(
                        at[:, :, c0a:], ps[:, :, c0a:], EXP,
                        scale=SCALE, bias=eb,
                    )
                elif last:
                    # split: halves the exp latency gating the finalize chain
                    nc.scalar.activation(
                        at[:, 0, :], ps[:, 0, :], EXP, scale=SCALE, bias=eb
                    )
                    nc.scalar.activation(
                        at[:, 1, :], ps[:, 1, :], EXP, scale=SCALE, bias=eb
                    )
                else:
                    nc.scalar.activation(at, ps, EXP, scale=SCALE, bias=eb)
                # masks first, then the PV matmuls (so the PE only waits on
                # exp+mask), then the Bsum adds behind them on the DVE
                c0s = []
                for t in range(2):
                    kb = 2 * j + t
                    i = kb - 4 * g
                    c0 = 128 * i if i > 0 else 0
                    c0s.append(c0)
                    if i >= 0:
                        # triangle mask on the diagonal 128-block; columns
                        # left of it are never computed or read
                        nc.vector.tensor_mul(
                            at[:, t, c0 : c0 + 128], at[:, t, c0 : c0 + 128], tri
                        )
                for t in range(2):
                    kb = 2 * j + t
                    st, sp = first and t == 0, last and t == 1
                    nc.tensor.matmul(
                        po[:, c0s[t]:],
                        v_t[:, kb, h * 128 : (h + 1) * 128],
                        at[:, t, c0s[t]:],
                        start=st, stop=sp,
                    )
                # Bsum accumulation on DVE (fp16 2x) replaces the
                # ones-matmul row sums; in the exp-paced late windows every
                # other t=1 add runs on the otherwise-idle Pool engine
                for t in range(2):
                    c0 = c0s[t]
                    if first and t == 0:
                        nc.vector.tensor_copy(bsum, at[:, 0, :])
                    elif g >= 2 and t == 1 and j % 2 == 1:
                        nc.gpsimd.tensor_add(
                            bsum[:, c0:], bsum[:, c0:], at[:, t, c0:]
                        )
                    else:
                        nc.vector.tensor_add(
                            bsum[:, c0:], bsum[:, c0:], at[:, t, c0:]
                        )
                if last:
                    # cross-partition reduce: PE ones-matmul for the very
                    # last stream (short tail latency), otherwise the idle
                    # Pool engine
                    if g == NQG - 1 and h == HPC - 1:
                        sums_ps = ps_sh.tile(
                            [128, 512], F32, tag="sh", name="sums_ps"
                        )
                        nc.tensor.matmul(
                            sums_ps, ones, bsum, start=True, stop=True
                        )
                        sums = pn.tile([128, 512], F32, tag="sums")
                        nc.vector.tensor_copy(sums, sums_ps)
                    else:
                        sums = pn.tile([128, 512], F32, tag="sums")
                        nc.gpsimd.partition_all_reduce(
                            sums, bsum, 128, bass_isa.ReduceOp.add
                        )
                    state[("sm", g, h)] = sums
                    # finalize one slot later: frees the po PSUM bank for
                    # the next stream with the Pool/ones latency hidden
                    pending.append((1, emit_finalize(g, h)))

            def emit_proj_lb(g, b):
                def run():
                    lb = g * 4 + b
                    lsl = slice(lb * 128, (lb + 1) * 128)
                    final = g == NQG - 1 and b == 3
                    ysb = py_.tile([128, D], F16, tag="ysb")
                    for eh in range(2):
                        pyp = ps_sh.tile([128, 512], F32, tag="sh", name="pyp")
                        for h in range(HPC):
                            nc.tensor.matmul(
                                pyp[:, :384],
                                o_t[:, h, lsl],
                                wo_sb[:, h, eh * 384 : (eh + 1) * 384],
                                start=(h == 0), stop=(h == HPC - 1),
                            )
                        if eh == 1 and g != 2:
                            # eh1 on ACT so the two halves' copies overlap;
                            # group 2's land in the exp-paced last window,
                            # where the DVE has more slack than the ACT
                            nc.scalar.activation(
                                ysb[:, 384:768], pyp[:, :384], IDENT, bias=0.0
                            )
                        else:
                            nc.vector.tensor_copy(
                                ysb[:, eh * 384 : (eh + 1) * 384], pyp[:, :384]
                            )
                        if final:
                            nc.sync.dma_start(
                                y_d[lsl, eh * 384 : (eh + 1) * 384],
                                ysb[:, eh * 384 : (eh + 1) * 384],
                            )
                    if not final:
                        nc.sync.dma_start(y_d[lsl, :], ysb)
                return run

            # track when each (g,h) stream's last batch appears so the
            # projection can be scheduled right after its group completes
            done_after = {}
            for idx, (g, h, j) in enumerate(batch_order):
                if j == last_j[(g, h)]:
                    done_after[(g, h)] = idx
            proj_at = {}
            for g in range(NQG):
                idx = max(done_after[(g, h)] for h in range(HPC))
                for b in range(4):
                    proj_at.setdefault(idx, []).append((g, b))

            def emit_unit(u):
                kind, g, i = u
                if kind == "qk":
                    emit_qk_unit(g, i)
                else:
                    emit_v_unit(g, i)

            for u in fill.get(-1, []):
                emit_unit(u)
            emit_S(*batch_order[0])
            nslots = len(batch_order)
            for mi in range(nslots):
                m = batch_order[mi]
                for u in fill.get(mi, []):
                    emit_unit(u)
                if mi + 1 < nslots:
                    emit_S(*batch_order[mi + 1])
                nxt = []
                for d, fn in pending:
                    if d <= 0:
                        fn()
                    else:
                        nxt.append((d - 1, fn))
                pending = nxt
                emit_rest(m)
                if mi in proj_at:
                    for d, (g, b) in enumerate(proj_at[mi]):
                        pending.append((2 + d, emit_proj_lb(g, b)))
            for d, fn in sorted(pending, key=lambda p: p[0]):
                fn()

    nc.compile()
    return nc


_NC_CACHE = {}


def _get_nc(L_=L):
    if L_ not in _NC_CACHE:
        _NC_CACHE[L_] = build_nc(L_)
    return _NC_CACHE[L_]


def run_sharded(inputs, L_=L, trace=False):
    """Shard inputs over 8 cores, run, return results object."""
    x = np.asarray(inputs["x_input"], dtype=np.float32)
    tri = (np.arange(128)[None, :] >= np.arange(128)[:, None]).astype(np.float16)
    trio = np.concatenate([tri, np.ones((128, 128), np.float16)], axis=1)
    eb = np.full((128, 1), EXP_BIAS, dtype=np.float32)
    in_maps = []
    for c in range(N_CORES):
        b, gslice = c // 2, slice((c % 2) * HG, (c % 2) * HG + HG)
        wqkv = np.concatenate(
            [
                np.asarray(inputs["Wq"], np.float32)[:, gslice],
                np.asarray(inputs["Wk"], np.float32)[:, gslice],
                np.asarray(inputs["Wv"], np.float32)[:, gslice],
            ],
            axis=1,
        ).astype(np.float16)
        in_maps.append(
            {
                "xt": np.ascontiguousarray(x[b].T.astype(np.float16)),
                "wqkv": np.ascontiguousarray(wqkv),
                "wo": np.ascontiguousarray(
                    np.asarray(inputs["Wo"], np.float32)[gslice, :].astype(np.float16)
                ),
                "bq": np.ascontiguousarray(
                    np.asarray(inputs["bq"], np.float32)[gslice]
                ),
                "bk": np.ascontiguousarray(
                    np.asarray(inputs["bk"], np.float32)[gslice]
                ),
                "tri": trio,
                "eb": eb,
            }
        )
    nc = _get_nc(L_)
    try:
        res = run_bass_kernel_spmd(nc, in_maps, list(range(N_CORES)), trace=trace)
    except Exception:
        # transient device faults (NRT_EXEC_UNIT_UNRECOVERABLE etc.): one retry
        res = run_bass_kernel_spmd(nc, in_maps, list(range(N_CORES)), trace=trace)
    return res


def kernel(**inputs) -> np.ndarray:
    res = run_sharded(inputs)
    # host-side unshard: sum the two head-group partials per batch; add the
    # bias terms that commute out of the device computation exactly:
    # softmax rows sum to 1, so  A @ (xWv + bv) Wo + bo = A(xWv)Wo + bv@Wo + bo
    bias = (
        np.asarray(inputs["bv"], np.float32) @ np.asarray(inputs["Wo"], np.float32)
        + np.asarray(inputs["bo"], np.float32)
    )
    out = np.empty((B, L, D), dtype=np.float32)
    for b in range(B):
        out[b] = (
            res.results[2 * b]["y"].astype(np.float32)
            + res.results[2 * b + 1]["y"].astype(np.float32)
            + bias
        )
    return out


# revision 16
# speedup vs baseline: 1.0499x; 1.0499x over previous
"""Causal self-attention for B=4, L=2048, D=768, H=6 on 8 TRN2 NeuronCores.

Sharding: 8 cores = 4 batches x 2 head-groups (3 heads / 384 hidden each).
All matmul operands are fp16 (host converts x/weights; ~0.5% rel err, well
inside the 2e-2 gate). Per core, for its (batch, head-group):

  x^T is uploaded pre-transposed (fp16), so no PE transposes at all.
  QT/KT = (Wq,k chunk)^T-stationary @ x^T-moving   [128d x L per head]
  V     = x^T-stationary @ Wv-moving               [L x 384]
  per head, per 512-wide q-group, per 128-key block (causal skip at 128
  granularity — fp16 runs 1 cyc/row at any width):
    S^T  = K_blk @ Q^T            (PE)
    A^T  = exp(S^T/sqrt(128) - 2) (ACT, fp16 out; -2 guards fp16 range)
    tri-mask on diagonal blocks   (DVE, fp16 2x mode)
    O^T += V_blk^T @ A^T          (PE, accumulated in PSUM)
    Bsum += A^T                   (DVE fp16 adds — replaces the row-sum
                                   ones-matmuls that used to burn PE time)
  sums  = partition_all_reduce(Bsum)  (GPSIMD/Pool — idle engine; the very
          last group instead uses a PE ones-matmul to cut tail latency)
  O^T   = po / sums                   (single DVE divide)
  Y_part = O @ Wo_slice               (PE, via O^T-stationary)

The QKV projections are interleaved into the attention stream: the PE
executes group g+1's QKV matmuls between group g's attention batches, so
the exp (ACT) latency is hidden behind projection work instead of stalling
the PE.  Host sums the two head-group partials per batch and adds
(bv @ Wo + bo); bq/bk are applied on-device (free per-partition bias in
the PSUM->SBUF copies). The exp -2 bias cancels in softmax normalization.
"""

import math

import numpy as np

import concourse.bacc as bacc
import concourse.mybir as mybir
import concourse.tile as tile
from concourse import bass_isa
from concourse.bass_utils import run_bass_kernel_spmd

F32 = mybir.dt.float32
F16 = mybir.dt.float16
EXP = mybir.ActivationFunctionType.Exp
IDENT = mybir.ActivationFunctionType.Identity
DIV = mybir.AluOpType.divide

B = 4
L = 2048
D = 768
HEADS = 6
HD = 128
HPC = 3          # heads per core
HG = HPC * HD    # 384: per-core slice of the hidden dim
CB = D // 128    # 6 contraction chunks
SCALE = 1.0 / math.sqrt(HD)
EXP_BIAS = -2.0  # exp(S*scale - 2): keeps A and its sums in fp16 range
N_CORES = 8


def build_nc(L_=L):
    """Build + compile the per-core Bass program (same program on all cores)."""
    NQG = L_ // 512   # 512-wide q groups

    nc = bacc.Bacc("TRN2", target_bir_lowering=False, debug=False)
    xt_d = nc.dram_tensor("xt", [D, L_], F16, kind="ExternalInput").ap()
    wqkv_d = nc.dram_tensor("wqkv", [D, 3 * HG], F16, kind="ExternalInput").ap()
    wo_d = nc.dram_tensor("wo", [HG, D], F16, kind="ExternalInput").ap()
    bq_d = nc.dram_tensor("bq", [HG], F32, kind="ExternalInput").ap()
    bk_d = nc.dram_tensor("bk", [HG], F32, kind="ExternalInput").ap()
    tri_d = nc.dram_tensor("tri", [128, 256], F16, kind="ExternalInput").ap()
    eb_d = nc.dram_tensor("eb", [128, 1], F32, kind="ExternalInput").ap()
    y_d = nc.dram_tensor("y", [L_, D], F16, kind="ExternalOutput").ap()

    with tile.TileContext(nc) as tc:
        with (
            tc.tile_pool(name="persist", bufs=1) as pp,
            tc.tile_pool(name="qkv_sb", bufs=1) as pqkv,
            tc.tile_pool(name="at_pool", bufs=8) as pat,
            tc.tile_pool(name="bsum_p", bufs=3) as pbs,
            tc.tile_pool(name="nrm_sb", bufs=3) as pn,
            tc.tile_pool(name="y_pool", bufs=3) as py_,
            tc.tile_pool(name="ps_s", bufs=2, space="PSUM") as ps_s,
            tc.tile_pool(name="ps_o", bufs=2, space="PSUM") as ps_o,
            tc.tile_pool(name="ps_sh", bufs=2, space="PSUM") as ps_sh,
        ):
            # tiny memset-fed matmul right at program start: begins the PE
            # p-state ramp clock ~3us before the first real matmul, so QKV
            # group 0 runs at full clock
            dseed = pp.tile([128, 2], F16)
            nc.vector.memset(dseed, 0)
            dmy = ps_sh.tile([128, 512], F32, tag="sh", name="dmy")
            nc.tensor.matmul(
                dmy[:1, :2], dseed[:, :1], dseed, start=True, stop=True
            )

            # constants go on the SWDGE (gpsimd) queue so the HWDGE queue's
            # first descriptors are the weight / x^T chunks the PE waits on
            eb = pp.tile([128, 1], F32)
            nc.gpsimd.dma_start(eb, eb_d)
            trio = pp.tile([128, 256], F16)
            nc.gpsimd.dma_start(trio, tri_d)
            bq_sb = pp.tile([128, HPC], F32)
            bk_sb = pp.tile([128, HPC], F32)
            nc.gpsimd.dma_start(bq_sb, bq_d.rearrange("(h p) -> p h", p=128))
            nc.gpsimd.dma_start(bk_sb, bk_d.rearrange("(h p) -> p h", p=128))
            # dummy exp: pulls the ACT Exp-table load off the critical path
            warm = pp.tile([1, 1], F32)
            nc.scalar.activation(warm, eb[:1, :], EXP, bias=eb[:1, :])

            q_t = pqkv.tile([128, HPC, L_], F16)   # Q^T: [d, (head, L)]
            k_t = pqkv.tile([128, HPC, L_], F16)   # K^T
            v_t = pqkv.tile([128, L_ // 128, HG], F16)  # V: [k-in-block, (block, hd)]
            o_t = pqkv.tile([128, HPC, L_], F16)   # O^T (normalized)
            xt = pqkv.tile([128, CB, L_], F16)     # x^T: [d-in-chunk, (chunk, L)]
            wqkv_sb = pqkv.tile([128, CB, 3 * HG], F16)
            wo_sb = pqkv.tile([128, HPC, D], F16)

            xt_r = xt_d.rearrange("(c p) l -> p c l", p=128)
            wqkv_r = wqkv_d.rearrange("(c p) d -> p c d", p=128)
            # interleave so the group-0 Q/K matmuls can start ~3us in: per
            # chunk c, the weight chunk then the first-512 x^T columns; the
            # first weight chunk is split so the very first Q matmul's
            # operands arrive as early as possible
            nc.sync.dma_start(wqkv_sb[:, 0, 0:HG], wqkv_r[:, 0, 0:HG])
            nc.sync.dma_start(xt[:, 0, 0:512], xt_r[:, 0, 0:512])
            nc.sync.dma_start(wqkv_sb[:, 0, HG:], wqkv_r[:, 0, HG:])
            for c in range(1, CB):
                nc.sync.dma_start(wqkv_sb[:, c, :], wqkv_r[:, c, :])
                nc.sync.dma_start(xt[:, c, 0:512], xt_r[:, c, 0:512])
            for g in range(1, 4):
                qsl = slice(g * 512, (g + 1) * 512)
                nc.sync.dma_start(xt[:, :, qsl], xt_r[:, :, qsl])
                if g == 1:
                    nc.sync.dma_start(
                        wo_sb, wo_d.rearrange("(h p) e -> p h e", p=128)
                    )

            tri = trio[:, 0:128]
            ones = trio[:, 128:256]

            # ---- QKV group 0: chunk-major so the PE rides the arriving
            # per-chunk DMAs without stalling (3 heads' q+k accumulate in
            # 6 PSUM banks at once; attention pools are idle this early) ----
            qk_ps = {
                0: (lambda t_: (t_[:, 0, :], t_[:, 1, :]))(
                    ps_s.tile([128, 2, 512], F32, tag="ps", name="qk0")
                ),
                1: (lambda t_: (t_[:, 0, :], t_[:, 1, :]))(
                    ps_s.tile([128, 2, 512], F32, tag="ps", name="qk1")
                ),
                2: (
                    ps_o.tile([128, 512], F32, tag="po", name="qk2q"),
                    ps_sh.tile([128, 512], F32, tag="sh", name="qk2k"),
                ),
            }
            for c in range(CB):
                for h in range(HPC):
                    pq, pk = qk_ps[h]
                    nc.tensor.matmul(
                        pq, wqkv_sb[:, c, h * 128 : (h + 1) * 128],
                        xt[:, c, 0:512],
                        start=(c == 0), stop=(c == CB - 1),
                    )
                    nc.tensor.matmul(
                        pk, wqkv_sb[:, c, HG + h * 128 : HG + (h + 1) * 128],
                        xt[:, c, 0:512],
                        start=(c == 0), stop=(c == CB - 1),
                    )
            for h in range(HPC):
                pq, pk = qk_ps[h]
                nc.scalar.activation(
                    q_t[:, h, 0:512], pq, IDENT, bias=bq_sb[:, h : h + 1]
                )
                nc.scalar.activation(
                    k_t[:, h, 0:512], pk, IDENT, bias=bk_sb[:, h : h + 1]
                )
            for b in range(4):
                pv = ps_sh.tile([128, 512], F32, tag="sh", name="pv")
                for c in range(CB):
                    nc.tensor.matmul(
                        pv[:, :HG], xt[:, c, b * 128 : (b + 1) * 128],
                        wqkv_sb[:, c, 2 * HG : 3 * HG],
                        start=(c == 0), stop=(c == CB - 1),
                    )
                nc.vector.tensor_copy(v_t[:, b, :], pv[:, :HG])

            # ---- interleaved QKV(g+1) / attention(g) stream ----

            def emit_qk_unit(g, h):
                qsl = slice(g * 512, (g + 1) * 512)
                pq = ps_sh.tile([128, 512], F32, tag="sh", name="pq")
                for c in range(CB):
                    nc.tensor.matmul(
                        pq, wqkv_sb[:, c, h * 128 : (h + 1) * 128], xt[:, c, qsl],
                        start=(c == 0), stop=(c == CB - 1),
                    )
                nc.scalar.activation(
                    q_t[:, h, qsl], pq, IDENT, bias=bq_sb[:, h : h + 1]
                )
                pk = ps_sh.tile([128, 512], F32, tag="sh", name="pk")
                for c in range(CB):
                    nc.tensor.matmul(
                        pk, wqkv_sb[:, c, HG + h * 128 : HG + (h + 1) * 128],
                        xt[:, c, qsl],
                        start=(c == 0), stop=(c == CB - 1),
                    )
                nc.scalar.activation(
                    k_t[:, h, qsl], pk, IDENT, bias=bk_sb[:, h : h + 1]
                )

            def emit_v_unit(g, b):
                lb = g * 4 + b
                pv = ps_sh.tile([128, 512], F32, tag="sh", name="pv")
                for c in range(CB):
                    nc.tensor.matmul(
                        pv[:, :HG], xt[:, c, lb * 128 : (lb + 1) * 128],
                        wqkv_sb[:, c, 2 * HG : 3 * HG],
                        start=(c == 0), stop=(c == CB - 1),
                    )
                nc.vector.tensor_copy(v_t[:, lb, :], pv[:, :HG])

            # attention batches: per (g,h), j indexes pairs of 128-key blocks
            flat = []
            win_start = {}
            for g in range(NQG):
                win_start[g] = len(flat)
                nb = 2 * (g + 1)
                for h in range(HPC):
                    for pos in range(nb):
                        flat.append((g, h, pos, pos == nb - 1, pos == 0))
            state = {}
            pending = []  # (delay, closure)

            def emit_S(m):
                g, h, j, last, first = flat[m]
                ps = ps_s.tile([128, 2, 512], F32, tag="ps")
                for t in range(2):
                    kb = 2 * j + t
                    i = kb - 4 * g
                    c0 = 128 * i if i > 0 else 0
                    nc.tensor.matmul(
                        ps[:, t, c0:],
                        k_t[:, h, kb * 128 : (kb + 1) * 128],
                        q_t[:, h, g * 512 + c0 : (g + 1) * 512],
                        start=True, stop=True,
                    )
                state[m] = ps

            def emit_rest(m):
                g, h, j, last, first = flat[m]
                ps = state.pop(m)
                if first:
                    state[("po", g, h)] = ps_o.tile(
                        [128, 512], F32, tag="po", name="po"
                    )
                    state[("bs", g, h)] = pbs.tile(
                        [128, 512], F16, tag="bs", name="bsum"
                    )
                po = state[("po", g, h)]
                bsum = state[("bs", g, h)]
                at = pat.tile([128, 2, 512], F16)
                diag = j >= 2 * g
                if diag:
                    # single strided call over both key blocks, starting at
                    # the first block's causal offset; the [c0a, c0b) sliver
                    # of t=1 exp's stale PSUM that nothing ever reads
                    c0a = 128 * (2 * j - 4 * g)
                    nc.scalar.activation(
                        at[:, :, c0a:], ps[:, :, c0a:], EXP,
                        scale=SCALE, bias=eb,
                    )
                elif last:
                    # split: halves the exp latency gating this group's
                    # finalize chain
                    nc.scalar.activation(
                        at[:, 0, :], ps[:, 0, :], EXP, scale=SCALE, bias=eb
                    )
                    nc.scalar.activation(
                        at[:, 1, :], ps[:, 1, :], EXP, scale=SCALE, bias=eb
                    )
                else:
                    nc.scalar.activation(at, ps, EXP, scale=SCALE, bias=eb)
                for t in range(2):
                    kb = 2 * j + t
                    i = kb - 4 * g
                    c0 = 128 * i if i > 0 else 0
                    if i >= 0:
                        # triangle mask on the diagonal 128-block; columns
                        # left of it are never computed or read
                        nc.vector.tensor_mul(
                            at[:, t, c0 : c0 + 128], at[:, t, c0 : c0 + 128], tri
                        )
                    # Bsum accumulation on DVE (fp16 2x) replaces the
                    # ones-matmul row sums
                    if first and t == 0:
                        nc.vector.tensor_copy(bsum, at[:, 0, :])
                    else:
                        nc.vector.tensor_add(
                            bsum[:, c0:], bsum[:, c0:], at[:, t, c0:]
                        )
                    st, sp = first and t == 0, last and t == 1
                    nc.tensor.matmul(
                        po[:, c0:],
                        v_t[:, kb, h * 128 : (h + 1) * 128],
                        at[:, t, c0:],
                        start=st, stop=sp,
                    )
                if last:
                    # cross-partition reduce launches immediately; the final
                    # group head uses a PE ones-matmul (short latency, and
                    # the PE has slack in the last window) while the rest go
                    # to the otherwise-idle Pool engine
                    if g == NQG - 1 and h == HPC - 1:
                        sums_ps = ps_sh.tile(
                            [128, 512], F32, tag="sh", name="sums_ps"
                        )
                        nc.tensor.matmul(
                            sums_ps, ones, bsum, start=True, stop=True
                        )
                        sums = pn.tile([128, 512], F32, tag="sums")
                        nc.vector.tensor_copy(sums, sums_ps)
                    else:
                        sums = pn.tile([128, 512], F32, tag="sums")
                        nc.gpsimd.partition_all_reduce(
                            sums, bsum, 128, bass_isa.ReduceOp.add
                        )
                    state[("sm", g, h)] = sums

            def emit_finalize(g, h):
                def run():
                    po = state.pop(("po", g, h))
                    state.pop(("bs", g, h))
                    sums = state.pop(("sm", g, h))
                    recip = pn.tile([128, 512], F32, tag="recip")
                    nc.vector.reciprocal(recip, sums)
                    nc.vector.tensor_mul(
                        o_t[:, h, g * 512 : (g + 1) * 512], po, recip
                    )
                return run

            def emit_proj_lb(g, b):
                def run():
                    lb = g * 4 + b
                    lsl = slice(lb * 128, (lb + 1) * 128)
                    final = g == NQG - 1 and b == 3
                    ysb = py_.tile([128, D], F16, tag="ysb")
                    for eh in range(2):
                        pyp = ps_sh.tile([128, 512], F32, tag="sh", name="pyp")
                        for h in range(HPC):
                            nc.tensor.matmul(
                                pyp[:, :384],
                                o_t[:, h, lsl],
                                wo_sb[:, h, eh * 384 : (eh + 1) * 384],
                                start=(h == 0), stop=(h == HPC - 1),
                            )
                        if eh == 0:
                            nc.vector.tensor_copy(ysb[:, 0:384], pyp[:, :384])
                        else:
                            nc.scalar.activation(
                                ysb[:, 384:768], pyp[:, :384], IDENT, bias=0.0
                            )
                        if final:
                            # split the very last store so its first half
                            # overlaps the second half's matmuls
                            nc.sync.dma_start(
                                y_d[lsl, eh * 384 : (eh + 1) * 384],
                                ysb[:, eh * 384 : (eh + 1) * 384],
                            )
                    if not final:
                        nc.sync.dma_start(y_d[lsl, :], ysb)
                return run

            # per-window QKV filler units: window g interleaves group g+1's
            # QKV work between group g's attention batches
            def window_units(g):
                if g + 1 >= NQG:
                    return []
                u = [("qk", g + 1, h) for h in range(HPC)]
                u += [("v", g + 1, b) for b in range(4)]
                return u

            def emit_unit(u):
                kind, g, i = u
                if kind == "qk":
                    emit_qk_unit(g, i)
                else:
                    emit_v_unit(g, i)

            emit_S(0)
            for g in range(NQG):
                units = window_units(g)
                nbatch = 6 * (g + 1)
                emitted_u = 0
                for bi in range(nbatch):
                    m = win_start[g] + bi
                    if m + 1 < len(flat):
                        emit_S(m + 1)
                    # spread this window's QKV units across its batches
                    want = (len(units) * (bi + 1)) // nbatch
                    while emitted_u < want:
                        emit_unit(units[emitted_u])
                        emitted_u += 1
                    nxt = []
                    for d, fn in pending:
                        if d <= 0:
                            fn()
                        else:
                            nxt.append((d - 1, fn))
                    pending = nxt
                    emit_rest(m)
                    _, h, j, last, first = flat[m]
                    if last:
                        pending.append((1, emit_finalize(g, h)))
                        if h == HPC - 1:
                            # spread the projection's L-blocks so the PSUM
                            # slot rotation hides each pyp's copy latency
                            for b in range(4):
                                pending.append((2 + b, emit_proj_lb(g, b)))
            for d, fn in sorted(pending, key=lambda p: p[0]):
                fn()

    nc.compile()
    return nc


_NC_CACHE = {}


def _get_nc(L_=L):
    if L_ not in _NC_CACHE:
        _NC_CACHE[L_] = build_nc(L_)
    return _NC_CACHE[L_]


def run_sharded(inputs, L_=L, trace=False):
    """Shard inputs over 8 cores, run, return results object."""
    x = np.asarray(inputs["x_input"], dtype=np.float32)
    tri = (np.arange(128)[None, :] >= np.arange(128)[:, None]).astype(np.float16)
    trio = np.concatenate([tri, np.ones((128, 128), np.float16)], axis=1)
    eb = np.full((128, 1), EXP_BIAS, dtype=np.float32)
    in_maps = []
    for c in range(N_CORES):
        b, gslice = c // 2, slice((c % 2) * HG, (c % 2) * HG + HG)
        wqkv = np.concatenate(
            [
                np.asarray(inputs["Wq"], np.float32)[:, gslice],
                np.asarray(inputs["Wk"], np.float32)[:, gslice],
                np.asarray(inputs["Wv"], np.float32)[:, gslice],
            ],
            axis=1,
        ).astype(np.float16)
        in_maps.append(
            {
                "xt": np.ascontiguousarray(x[b].T.astype(np.float16)),
                "wqkv": np.ascontiguousarray(wqkv),
                "wo": np.ascontiguousarray(
                    np.asarray(inputs["Wo"], np.float32)[gslice, :].astype(np.float16)
                ),
                "bq": np.ascontiguousarray(
                    np.asarray(inputs["bq"], np.float32)[gslice]
                ),
                "bk": np.ascontiguousarray(
                    np.asarray(inputs["bk"], np.float32)[gslice]
                ),
                "tri": trio,
                "eb": eb,
            }
        )
    nc = _get_nc(L_)
    try:
        res = run_bass_kernel_spmd(nc, in_maps, list(range(N_CORES)), trace=trace)
    except Exception:
        # transient device faults (NRT_EXEC_UNIT_UNRECOVERABLE etc.): one retry
        res = run_bass_kernel_spmd(nc, in_maps, list(range(N_CORES)), trace=trace)
    return res


def kernel(**inputs) -> np.ndarray:
    res = run_sharded(inputs)
    # host-side unshard: sum the two head-group partials per batch; add the
    # bias terms that commute out of the device computation exactly:
    # softmax rows sum to 1, so  A @ (xWv + bv) Wo + bo = A(xWv)Wo + bv@Wo + bo
    bias = (
        np.asarray(inputs["bv"], np.float32) @ np.asarray(inputs["Wo"], np.float32)
        + np.asarray(inputs["bo"], np.float32)
    )
    out = np.empty((B, L, D), dtype=np.float32)
    for b in range(B):
        out[b] = (
            res.results[2 * b]["y"].astype(np.float32)
            + res.results[2 * b + 1]["y"].astype(np.float32)
            + bias
        )
    return out


# revision 17
# speedup vs baseline: 1.0511x; 1.0012x over previous
"""Causal self-attention for B=4, L=2048, D=768, H=6 on 8 TRN2 NeuronCores.

Sharding: 8 cores = 4 batches x 2 head-groups (3 heads / 384 hidden each).
All matmul operands are fp16 (host converts x/weights; ~0.5% rel err, well
inside the 2e-2 gate). Per core, for its (batch, head-group):

  x^T is uploaded pre-transposed (fp16), so no PE transposes at all.
  QT/KT = (Wq,k chunk)^T-stationary @ x^T-moving   [128d x L per head]
  V     = x^T-stationary @ Wv-moving               [L x 384]
  per head, per 512-wide q-group, per 128-key block (causal skip at 128
  granularity — fp16 runs 1 cyc/row at any width):
    S^T  = K_blk @ Q^T            (PE)
    A^T  = exp(S^T/sqrt(128) - 2) (ACT, fp16 out; -2 guards fp16 range)
    tri-mask on diagonal blocks   (DVE, fp16 2x mode)
    O^T += V_blk^T @ A^T          (PE, accumulated in PSUM)
    Bsum += A^T                   (DVE fp16 adds — replaces the row-sum
                                   ones-matmuls that used to burn PE time)
  sums  = partition_all_reduce(Bsum)  (GPSIMD/Pool — idle engine; the very
          last group instead uses a PE ones-matmul to cut tail latency)
  O^T   = po / sums                   (single DVE divide)
  Y_part = O @ Wo_slice               (PE, via O^T-stationary)

The QKV projections are interleaved into the attention stream: the PE
executes group g+1's QKV matmuls between group g's attention batches, so
the exp (ACT) latency is hidden behind projection work instead of stalling
the PE.  Host sums the two head-group partials per batch and adds
(bv @ Wo + bo); bq/bk are applied on-device (free per-partition bias in
the PSUM->SBUF copies). The exp -2 bias cancels in softmax normalization.
"""

import math

import numpy as np

import concourse.bacc as bacc
import concourse.mybir as mybir
import concourse.tile as tile
from concourse import bass_isa
from concourse.bass_utils import run_bass_kernel_spmd

F32 = mybir.dt.float32
F16 = mybir.dt.float16
EXP = mybir.ActivationFunctionType.Exp
IDENT = mybir.ActivationFunctionType.Identity
DIV = mybir.AluOpType.divide

B = 4
L = 2048
D = 768
HEADS = 6
HD = 128
HPC = 3          # heads per core
HG = HPC * HD    # 384: per-core slice of the hidden dim
CB = D // 128    # 6 contraction chunks
SCALE = 1.0 / math.sqrt(HD)
EXP_BIAS = -2.0  # exp(S*scale - 2): keeps A and its sums in fp16 range
N_CORES = 8


def build_nc(L_=L):
    """Build + compile the per-core Bass program (same program on all cores)."""
    NQG = L_ // 512   # 512-wide q groups

    nc = bacc.Bacc("TRN2", target_bir_lowering=False, debug=False)
    xt_d = nc.dram_tensor("xt", [D, L_], F16, kind="ExternalInput").ap()
    wqkv_d = nc.dram_tensor("wqkv", [D, 3 * HG], F16, kind="ExternalInput").ap()
    wo_d = nc.dram_tensor("wo", [HG, D], F16, kind="ExternalInput").ap()
    bq_d = nc.dram_tensor("bq", [HG], F32, kind="ExternalInput").ap()
    bk_d = nc.dram_tensor("bk", [HG], F32, kind="ExternalInput").ap()
    tri_d = nc.dram_tensor("tri", [128, 256], F16, kind="ExternalInput").ap()
    eb_d = nc.dram_tensor("eb", [128, 1], F32, kind="ExternalInput").ap()
    y_d = nc.dram_tensor("y", [L_, D], F16, kind="ExternalOutput").ap()

    with tile.TileContext(nc) as tc:
        with (
            tc.tile_pool(name="persist", bufs=1) as pp,
            tc.tile_pool(name="qkv_sb", bufs=1) as pqkv,
            tc.tile_pool(name="at_pool", bufs=8) as pat,
            tc.tile_pool(name="bsum_p", bufs=3) as pbs,
            tc.tile_pool(name="nrm_sb", bufs=3) as pn,
            tc.tile_pool(name="y_pool", bufs=3) as py_,
            tc.tile_pool(name="ps_s", bufs=2, space="PSUM") as ps_s,
            tc.tile_pool(name="ps_o", bufs=2, space="PSUM") as ps_o,
            tc.tile_pool(name="ps_sh", bufs=2, space="PSUM") as ps_sh,
        ):
            # tiny memset-fed matmul right at program start: begins the PE
            # p-state ramp clock ~3us before the first real matmul, so QKV
            # group 0 runs at full clock
            dseed = pp.tile([128, 2], F16)
            nc.vector.memset(dseed, 0)
            dmy = ps_sh.tile([128, 512], F32, tag="sh", name="dmy")
            nc.tensor.matmul(
                dmy[:1, :2], dseed[:, :1], dseed, start=True, stop=True
            )

            # constants go on the SWDGE (gpsimd) queue so the HWDGE queue's
            # first descriptors are the weight / x^T chunks the PE waits on
            eb = pp.tile([128, 1], F32)
            nc.gpsimd.dma_start(eb, eb_d)
            trio = pp.tile([128, 256], F16)
            nc.gpsimd.dma_start(trio, tri_d)
            bq_sb = pp.tile([128, HPC], F32)
            bk_sb = pp.tile([128, HPC], F32)
            nc.gpsimd.dma_start(bq_sb, bq_d.rearrange("(h p) -> p h", p=128))
            nc.gpsimd.dma_start(bk_sb, bk_d.rearrange("(h p) -> p h", p=128))
            # dummy exp: pulls the ACT Exp-table load off the critical path
            warm = pp.tile([1, 1], F32)
            nc.scalar.activation(warm, eb[:1, :], EXP, bias=eb[:1, :])

            q_t = pqkv.tile([128, HPC, L_], F16)   # Q^T: [d, (head, L)]
            k_t = pqkv.tile([128, HPC, L_], F16)   # K^T
            v_t = pqkv.tile([128, L_ // 128, HG], F16)  # V: [k-in-block, (block, hd)]
            o_t = pqkv.tile([128, HPC, L_], F16)   # O^T (normalized)
            xt = pqkv.tile([128, CB, L_], F16)     # x^T: [d-in-chunk, (chunk, L)]
            wqkv_sb = pqkv.tile([128, CB, 3 * HG], F16)
            wo_sb = pqkv.tile([128, HPC, D], F16)

            xt_r = xt_d.rearrange("(c p) l -> p c l", p=128)
            wqkv_r = wqkv_d.rearrange("(c p) d -> p c d", p=128)
            # interleave so the group-0 Q/K matmuls can start ~3us in: per
            # chunk c, the weight chunk then the first-512 x^T columns; the
            # first weight chunk is split so the very first Q matmul's
            # operands arrive as early as possible
            nc.sync.dma_start(wqkv_sb[:, 0, 0:HG], wqkv_r[:, 0, 0:HG])
            nc.sync.dma_start(xt[:, 0, 0:512], xt_r[:, 0, 0:512])
            nc.sync.dma_start(wqkv_sb[:, 0, HG:], wqkv_r[:, 0, HG:])
            for c in range(1, CB):
                nc.sync.dma_start(wqkv_sb[:, c, :], wqkv_r[:, c, :])
                nc.sync.dma_start(xt[:, c, 0:512], xt_r[:, c, 0:512])
            for c in range(CB):
                nc.sync.dma_start(xt[:, c, 512:L_], xt_r[:, c, 512:L_])
            nc.sync.dma_start(wo_sb, wo_d.rearrange("(h p) e -> p h e", p=128))

            tri = trio[:, 0:128]
            ones = trio[:, 128:256]

            # ---- QKV group 0: chunk-major so the PE rides the arriving
            # per-chunk DMAs without stalling (3 heads' q+k accumulate in
            # 6 PSUM banks at once; attention pools are idle this early) ----
            qk_ps = {
                0: (lambda t_: (t_[:, 0, :], t_[:, 1, :]))(
                    ps_s.tile([128, 2, 512], F32, tag="ps", name="qk0")
                ),
                1: (lambda t_: (t_[:, 0, :], t_[:, 1, :]))(
                    ps_s.tile([128, 2, 512], F32, tag="ps", name="qk1")
                ),
                2: (
                    ps_o.tile([128, 512], F32, tag="po", name="qk2q"),
                    ps_sh.tile([128, 512], F32, tag="sh", name="qk2k"),
                ),
            }
            for c in range(CB):
                for h in range(HPC):
                    pq, pk = qk_ps[h]
                    nc.tensor.matmul(
                        pq, wqkv_sb[:, c, h * 128 : (h + 1) * 128],
                        xt[:, c, 0:512],
                        start=(c == 0), stop=(c == CB - 1),
                    )
                    nc.tensor.matmul(
                        pk, wqkv_sb[:, c, HG + h * 128 : HG + (h + 1) * 128],
                        xt[:, c, 0:512],
                        start=(c == 0), stop=(c == CB - 1),
                    )
            for h in range(HPC):
                pq, pk = qk_ps[h]
                nc.scalar.activation(
                    q_t[:, h, 0:512], pq, IDENT, bias=bq_sb[:, h : h + 1]
                )
                nc.scalar.activation(
                    k_t[:, h, 0:512], pk, IDENT, bias=bk_sb[:, h : h + 1]
                )
            for b in range(4):
                pv = ps_sh.tile([128, 512], F32, tag="sh", name="pv")
                for c in range(CB):
                    nc.tensor.matmul(
                        pv[:, :HG], xt[:, c, b * 128 : (b + 1) * 128],
                        wqkv_sb[:, c, 2 * HG : 3 * HG],
                        start=(c == 0), stop=(c == CB - 1),
                    )
                nc.vector.tensor_copy(v_t[:, b, :], pv[:, :HG])

            # ---- interleaved QKV(g+1) / attention(g) stream ----

            def emit_qk_unit(g, h):
                qsl = slice(g * 512, (g + 1) * 512)
                pq = ps_sh.tile([128, 512], F32, tag="sh", name="pq")
                for c in range(CB):
                    nc.tensor.matmul(
                        pq, wqkv_sb[:, c, h * 128 : (h + 1) * 128], xt[:, c, qsl],
                        start=(c == 0), stop=(c == CB - 1),
                    )
                nc.scalar.activation(
                    q_t[:, h, qsl], pq, IDENT, bias=bq_sb[:, h : h + 1]
                )
                pk = ps_sh.tile([128, 512], F32, tag="sh", name="pk")
                for c in range(CB):
                    nc.tensor.matmul(
                        pk, wqkv_sb[:, c, HG + h * 128 : HG + (h + 1) * 128],
                        xt[:, c, qsl],
                        start=(c == 0), stop=(c == CB - 1),
                    )
                nc.scalar.activation(
                    k_t[:, h, qsl], pk, IDENT, bias=bk_sb[:, h : h + 1]
                )

            def emit_v_unit(g, b):
                lb = g * 4 + b
                pv = ps_sh.tile([128, 512], F32, tag="sh", name="pv")
                for c in range(CB):
                    nc.tensor.matmul(
                        pv[:, :HG], xt[:, c, lb * 128 : (lb + 1) * 128],
                        wqkv_sb[:, c, 2 * HG : 3 * HG],
                        start=(c == 0), stop=(c == CB - 1),
                    )
                nc.vector.tensor_copy(v_t[:, lb, :], pv[:, :HG])

            # attention batches: per (g,h), j indexes pairs of 128-key blocks
            flat = []
            win_start = {}
            for g in range(NQG):
                win_start[g] = len(flat)
                nb = 2 * (g + 1)
                for h in range(HPC):
                    for pos in range(nb):
                        flat.append((g, h, pos, pos == nb - 1, pos == 0))
            state = {}
            pending = []  # (delay, closure)

            def emit_S(m):
                g, h, j, last, first = flat[m]
                ps = ps_s.tile([128, 2, 512], F32, tag="ps")
                for t in range(2):
                    kb = 2 * j + t
                    i = kb - 4 * g
                    c0 = 128 * i if i > 0 else 0
                    nc.tensor.matmul(
                        ps[:, t, c0:],
                        k_t[:, h, kb * 128 : (kb + 1) * 128],
                        q_t[:, h, g * 512 + c0 : (g + 1) * 512],
                        start=True, stop=True,
                    )
                state[m] = ps

            def emit_rest(m):
                g, h, j, last, first = flat[m]
                ps = state.pop(m)
                if first:
                    state[("po", g, h)] = ps_o.tile(
                        [128, 512], F32, tag="po", name="po"
                    )
                    state[("bs", g, h)] = pbs.tile(
                        [128, 512], F16, tag="bs", name="bsum"
                    )
                po = state[("po", g, h)]
                bsum = state[("bs", g, h)]
                at = pat.tile([128, 2, 512], F16)
                diag = j >= 2 * g
                if diag:
                    # single strided call over both key blocks, starting at
                    # the first block's causal offset; the [c0a, c0b) sliver
                    # of t=1 exp's stale PSUM that nothing ever reads
                    c0a = 128 * (2 * j - 4 * g)
                    nc.scalar.activation(
                        at[:, :, c0a:], ps[:, :, c0a:], EXP,
                        scale=SCALE, bias=eb,
                    )
                elif last:
                    # split: halves the exp latency gating this group's
                    # finalize chain
                    nc.scalar.activation(
                        at[:, 0, :], ps[:, 0, :], EXP, scale=SCALE, bias=eb
                    )
                    nc.scalar.activation(
                        at[:, 1, :], ps[:, 1, :], EXP, scale=SCALE, bias=eb
                    )
                else:
                    nc.scalar.activation(at, ps, EXP, scale=SCALE, bias=eb)
                for t in range(2):
                    kb = 2 * j + t
                    i = kb - 4 * g
                    c0 = 128 * i if i > 0 else 0
                    if i >= 0:
                        # triangle mask on the diagonal 128-block; columns
                        # left of it are never computed or read
                        nc.vector.tensor_mul(
                            at[:, t, c0 : c0 + 128], at[:, t, c0 : c0 + 128], tri
                        )
                    # Bsum accumulation on DVE (fp16 2x) replaces the
                    # ones-matmul row sums
                    if first and t == 0:
                        nc.vector.tensor_copy(bsum, at[:, 0, :])
                    else:
                        nc.vector.tensor_add(
                            bsum[:, c0:], bsum[:, c0:], at[:, t, c0:]
                        )
                    st, sp = first and t == 0, last and t == 1
                    nc.tensor.matmul(
                        po[:, c0:],
                        v_t[:, kb, h * 128 : (h + 1) * 128],
                        at[:, t, c0:],
                        start=st, stop=sp,
                    )
                if last:
                    # cross-partition reduce launches immediately; the final
                    # group head uses a PE ones-matmul (short latency, and
                    # the PE has slack in the last window) while the rest go
                    # to the otherwise-idle Pool engine
                    if g == NQG - 1 and h == HPC - 1:
                        sums_ps = ps_sh.tile(
                            [128, 512], F32, tag="sh", name="sums_ps"
                        )
                        nc.tensor.matmul(
                            sums_ps, ones, bsum, start=True, stop=True
                        )
                        sums = pn.tile([128, 512], F32, tag="sums")
                        nc.vector.tensor_copy(sums, sums_ps)
                    else:
                        sums = pn.tile([128, 512], F32, tag="sums")
                        nc.gpsimd.partition_all_reduce(
                            sums, bsum, 128, bass_isa.ReduceOp.add
                        )
                    state[("sm", g, h)] = sums

            def emit_finalize(g, h):
                def run():
                    po = state.pop(("po", g, h))
                    state.pop(("bs", g, h))
                    sums = state.pop(("sm", g, h))
                    recip = pn.tile([128, 512], F32, tag="recip")
                    nc.vector.reciprocal(recip, sums)
                    nc.vector.tensor_mul(
                        o_t[:, h, g * 512 : (g + 1) * 512], po, recip
                    )
                return run

            def emit_proj_lb(g, b):
                def run():
                    lb = g * 4 + b
                    lsl = slice(lb * 128, (lb + 1) * 128)
                    final = g == NQG - 1 and b == 3
                    ysb = py_.tile([128, D], F16, tag="ysb")
                    for eh in range(2):
                        pyp = ps_sh.tile([128, 512], F32, tag="sh", name="pyp")
                        for h in range(HPC):
                            nc.tensor.matmul(
                                pyp[:, :384],
                                o_t[:, h, lsl],
                                wo_sb[:, h, eh * 384 : (eh + 1) * 384],
                                start=(h == 0), stop=(h == HPC - 1),
                            )
                        if eh == 0:
                            nc.vector.tensor_copy(ysb[:, 0:384], pyp[:, :384])
                        else:
                            nc.scalar.activation(
                                ysb[:, 384:768], pyp[:, :384], IDENT, bias=0.0
                            )
                        if final:
                            # split the very last store so its first half
                            # overlaps the second half's matmuls
                            nc.sync.dma_start(
                                y_d[lsl, eh * 384 : (eh + 1) * 384],
                                ysb[:, eh * 384 : (eh + 1) * 384],
                            )
                    if not final:
                        nc.sync.dma_start(y_d[lsl, :], ysb)
                return run

            # per-window QKV filler units: window g interleaves group g+1's
            # QKV work between group g's attention batches
            def window_units(g):
                if g + 1 >= NQG:
                    return []
                u = [("qk", g + 1, h) for h in range(HPC)]
                u += [("v", g + 1, b) for b in range(4)]
                return u

            def emit_unit(u):
                kind, g, i = u
                if kind == "qk":
                    emit_qk_unit(g, i)
                else:
                    emit_v_unit(g, i)

            emit_S(0)
            for g in range(NQG):
                units = window_units(g)
                nbatch = 6 * (g + 1)
                emitted_u = 0
                for bi in range(nbatch):
                    m = win_start[g] + bi
                    if m + 1 < len(flat):
                        emit_S(m + 1)
                    # spread this window's QKV units across its batches
                    want = (len(units) * (bi + 1)) // nbatch
                    while emitted_u < want:
                        emit_unit(units[emitted_u])
                        emitted_u += 1
                    nxt = []
                    for d, fn in pending:
                        if d <= 0:
                            fn()
                        else:
                            nxt.append((d - 1, fn))
                    pending = nxt
                    emit_rest(m)
                    _, h, j, last, first = flat[m]
                    if last:
                        pending.append((1, emit_finalize(g, h)))
                        if h == HPC - 1:
                            # spread the projection's L-blocks so the PSUM
                            # slot rotation hides each pyp's copy latency
                            for b in range(4):
                                pending.append((2 + b, emit_proj_lb(g, b)))
            for d, fn in sorted(pending, key=lambda p: p[0]):
                fn()

    nc.compile()
    return nc


_NC_CACHE = {}


def _get_nc(L_=L):
    if L_ not in _NC_CACHE:
        _NC_CACHE[L_] = build_nc(L_)
    return _NC_CACHE[L_]


def run_sharded(inputs, L_=L, trace=False):
    """Shard inputs over 8 cores, run, return results object."""
    x = np.asarray(inputs["x_input"], dtype=np.float32)
    tri = (np.arange(128)[None, :] >= np.arange(128)[:, None]).astype(np.float16)
    trio = np.concatenate([tri, np.ones((128, 128), np.float16)], axis=1)
    eb = np.full((128, 1), EXP_BIAS, dtype=np.float32)
    in_maps = []
    for c in range(N_CORES):
        b, gslice = c // 2, slice((c % 2) * HG, (c % 2) * HG + HG)
        wqkv = np.concatenate(
            [
                np.asarray(inputs["Wq"], np.float32)[:, gslice],
                np.asarray(inputs["Wk"], np.float32)[:, gslice],
                np.asarray(inputs["Wv"], np.float32)[:, gslice],
            ],
            axis=1,
        ).astype(np.float16)
        in_maps.append(
            {
                "xt": np.ascontiguousarray(x[b].T.astype(np.float16)),
                "wqkv": np.ascontiguousarray(wqkv),
                "wo": np.ascontiguousarray(
                    np.asarray(inputs["Wo"], np.float32)[gslice, :].astype(np.float16)
                ),
                "bq": np.ascontiguousarray(
                    np.asarray(inputs["bq"], np.float32)[gslice]
                ),
                "bk": np.ascontiguousarray(
                    np.asarray(inputs["bk"], np.float32)[gslice]
                ),
                "tri": trio,
                "eb": eb,
            }
        )
    nc = _get_nc(L_)
    try:
        res = run_bass_kernel_spmd(nc, in_maps, list(range(N_CORES)), trace=trace)
    except Exception:
        # transient device faults (NRT_EXEC_UNIT_UNRECOVERABLE etc.): one retry
        res = run_bass_kernel_spmd(nc, in_maps, list(range(N_CORES)), trace=trace)
    return res


def kernel(**inputs) -> np.ndarray:
    res = run_sharded(inputs)
    # host-side unshard: sum the two head-group partials per batch; add the
    # bias terms that commute out of the device computation exactly:
    # softmax rows sum to 1, so  A @ (xWv + bv) Wo + bo = A(xWv)Wo + bv@Wo + bo
    bias = (
        np.asarray(inputs["bv"], np.float32) @ np.asarray(inputs["Wo"], np.float32)
        + np.asarray(inputs["bo"], np.float32)
    )
    out = np.empty((B, L, D), dtype=np.float32)
    for b in range(B):
        out[b] = (
            res.results[2 * b]["y"].astype(np.float32)
            + res.results[2 * b + 1]["y"].astype(np.float32)
            + bias
        )
    return out
